# revision 38
# baseline (speedup 1.0000x reference)
"""Polyphase 2x upsample (scatter into one of 4 phases per batch) + circular
3x3 binomial blur, distributed over 8 TRN2 NeuronCores (data-parallel over
batch: 2 batches per core).

Math: with phase p per batch, r = p % 2, c = p // 2, the reference scatters
x[i,j] to y1[2i+r, 2j+c] (zeros elsewhere) and then blurs with
outer([1,2,1],[1,2,1])/16 under circular padding. The output decomposes into
4 parity classes (all indices mod 128, mod 64 inside a pair):
  out[2i+r,   2j+c]   = x[i,j] / 4                    (A sites)
  out[2i+r,   2k+1+c] = (x[i,k] + x[i,k+1]) / 8       (H sites)
  out[2i+1+r, 2j+c]   = (x[i,j] + x[i+1,j]) / 8       (V sites)
  out[2i+1+r, 2k+1+c] = sum of the 4 neighbours / 16  (D sites)
All multiplies are powers of two (exact in fp32).

Memory-bound: 40 MiB/core of HBM traffic (8 read + 32 write), reads and
writes sharing one ~365 GB/s per-core cap => ~116us of saturated DMA is the
floor, plus ~9us of fixed NEFF/engine-init preamble. Schedule principles
(all measured on trn2):
 - Quarter-granularity store pipeline: each (batch, channel-half) chunk's
   128 output rows are produced in four ~32-row tiles, each stored the
   moment its sites complete; the first store issues at ~13us instead of
   ~37us (whole-chunk granularity), which removes the DMA hole between the
   end of the input-load stream and the first store.
 - Input loads are compressed into the FIRST ~25us (chunk 0's on ACT,
   chunks 1-2's upfront on SP, chunk 3's prefetched mid-chunk-1 from ACT):
   sustained load/store overlap skews SDMA ring 15 ~15% slow (its AXI port
   also serves DGE descriptor traffic), which shows up as a multi-us solo
   ring-15 drain tail after every balanced ring has finished. Keeping the
   overlap window short keeps ring 15's excess small. (Measured: overlap
   across the whole kernel costs ring 15 ~+15us of busy time regardless of
   load queue (SP vs ACT), store count (12 vs 18), or store shape
   (31/32/33 vs 64/63/1-row).)
 - SP issues only stores (plus the c1/c2 load issues that complete before
   the first store is data-ready): a dma_start costs ~850ns of issue time
   on its queue, so 16 load issues ahead of the store If would push the
   first store out by ~14us.
 - Strided-row DMA stores (per-row 512B descriptors) cost ~36% more HBM
   time than contiguous stores; all stores are contiguous row ranges.
 - GPSIMD software tensor ops contend with DVE for SBUF; Pool does no
   compute here. tensor_tensor_reduce faults the runtime; use adds.
 - HWDGE dma_start exists only on SP and ACT queues.

SPMD phase handling (one NEFF for all 8 cores):
 - The column phase bit c selects between two fully static write layouts
   via a runtime 2-arm If per chunk. All tiles are allocated OUTSIDE the
   If; both arms touch the same tiles with identical op counts. Pool-slot
   recycling must only happen ACROSS Ifs (slot release accounting for
   readers inside If arms reconciles at the If merge; reacquiring within
   the same If deadlocks).
 - The row shift r is folded into the output DMA's DRAM row offsets via a
   2-arm If on SP: static starts in both arms, so Tile proves all stores
   of a chunk hit disjoint DRAM rows and they drain in parallel.
 - skip_runtime_bounds_check everywhere: the emitted software assert
   instruction faults this runtime.
"""

import sys

for _p in ("/opt/trn_rl_repo",):
    if _p not in sys.path:
        sys.path.insert(0, _p)

import numpy as np

B, C, N = 16, 256, 64
M = 2 * N
NCORES = 8
NB = B // NCORES  # batches per core

_NC_CACHE = None


def _build_nc():
    import concourse.bacc as bacc
    import concourse.bass as bass
    import concourse.mybir as mybir
    import concourse.tile as tile

    f32 = mybir.dt.float32
    bf16 = mybir.dt.bfloat16
    i32 = mybir.dt.int32
    add = mybir.AluOpType.add
    ET = mybir.EngineType

    # Bacc (not plain Bass): its finalize() runs generate_event_semaphores,
    # which splits multi-wait instructions — this walrus build allows at
    # most one attached semaphore wait per instruction.
    nc = bacc.Bacc("TRN2", target_bir_lowering=False, debug=False, num_devices=NCORES)
    inp = nc.dram_tensor("inp", [NB, C, N, N], f32, kind="ExternalInput")
    offs = nc.dram_tensor("offs", [1, 16], i32, kind="ExternalInput")
    out = nc.dram_tensor("out", [NB, C, M, M], f32, kind="ExternalOutput")

    chunks = [(b, h) for b in range(NB) for h in range(C // 128)]

    with tile.TileContext(nc) as tc:
        with (
            tc.tile_pool(name="offp", bufs=1) as offp,
            tc.tile_pool(name="xp", bufs=12) as xp,
            tc.tile_pool(name="t16p", bufs=1) as t16p,
            tc.tile_pool(name="x8p", bufs=1) as x8p,
            tc.tile_pool(name="svp", bufs=1) as svp,
            tc.tile_pool(name="op", bufs=2) as op,
        ):
            def alloc_x(ci):
                b, h = chunks[ci]
                return [
                    xp.tile([128, 16, N], f32, tag="x", name=f"x_{b}_{h}_{j}")
                    for j in range(4)
                ]

            def issue_loads(ci, tiles, eng):
                b, h = chunks[ci]
                for j in range(4):
                    eng.dma_start(
                        tiles[j][:, :, :],
                        inp[b, 128 * h : 128 * (h + 1), 16 * j : 16 * j + 16],
                    )

            # ACT: offs (tiny) then chunk 0's loads, then the cv reg-loads
            # (which block ACT until offs lands — the input loads must
            # already be in flight). SP: chunks 1-2's loads upfront (done
            # issuing by ~13us, before the first store is data-ready),
            # then the rv reg-loads.
            offs_t = offp.tile([1, 16], i32)
            nc.scalar.dma_start(offs_t[:, :], offs[:, :])
            all_xs = [alloc_x(ci) for ci in range(len(chunks))]
            issue_loads(0, all_xs[0], nc.scalar)
            issue_loads(1, all_xs[1], nc.sync)
            issue_loads(2, all_xs[2], nc.sync)

            # per batch: [cv, rv] at offs[0, 8*b + k]
            val = {}
            for b in range(NB):
                for k, name, engs in (
                    (0, "cv", (ET.DVE, ET.Activation)),
                    (1, "rv", (ET.SP,)),
                ):
                    val[(b, name)] = nc.values_load(
                        offs_t[0:1, 8 * b + k : 8 * b + k + 1],
                        engines=list(engs),
                        min_val=0,
                        max_val=1,
                        skip_runtime_bounds_check=True,
                    )

            # Per-quarter output row groups (output row index before r shift):
            #   q0 -> rows [0,31)   : A/H at local even rows, V/D odd
            #   q1 -> rows [31,63)  : V/D at local even rows, A/H odd
            #   q2 -> rows [63,95)  : V/D even, A/H odd
            #   q3 -> rows [95,128) : V/D even, A/H odd, local row 32 = pair 63
            # Quarter j's A/H rows read x8 rows [16j,16j+16); its V/D rows
            # read Sv pairs (q0: [0,15), q1: [15,31), q2: [31,47),
            # q3: [47,63) plus the wrap pair 63 at Sv row 63). These spans
            # only need t16 rows <= 16j+15, so no quarter waits on a later
            # load.
            def compute_chunk(ci, xs, t16, x8, Sv, os, c):
                if c == 0:
                    a_cols = slice(0, 128, 2)
                    hm_cols = slice(1, 127, 2)
                    hw_col = 127
                    v_cols = slice(0, 128, 2)
                    dm_cols = slice(1, 127, 2)
                    dw_col = 127
                else:
                    a_cols = slice(1, 128, 2)
                    hm_cols = slice(2, 127, 2)
                    hw_col = 0
                    v_cols = slice(1, 128, 2)
                    dm_cols = slice(2, 127, 2)
                    dw_col = 0
                for j in range(4):
                    if ci == 1 and j == 2:
                        # chunk 3's loads: issued mid-chunk-1 from ACT; the
                        # issue's WAR conflict (xp bufs=12) is chunk 0's
                        # tiles, whose readers ran in chunk 0's If — one If
                        # back, so the slot accounting has reconciled.
                        issue_loads(3, all_xs[3], nc.scalar)
                    xq, o = xs[j], os[j]
                    hr = slice(16 * j, 16 * j + 16)
                    # t16 = x/16 feeds Sv; x8 = x/8 feeds A and H.
                    nc.scalar.mul(t16[:, hr, :], xq[:, :, :], 0.0625)
                    nc.scalar.mul(x8[:, hr, :], xq[:, :, :], 0.125)
                    # Sv pairs needed by this quarter's V/D rows
                    if j == 0:
                        pr = slice(0, 15)
                        ah = slice(0, 31, 2)   # 16 rows
                        vd = slice(1, 30, 2)   # 15 rows
                    else:
                        pr = slice(16 * j - 1, 16 * j + 15)
                        ah = slice(1, 32, 2)   # 16 rows
                        vd = slice(0, 31, 2)   # 16 rows
                    nc.vector.tensor_tensor(
                        Sv[:, pr, :],
                        t16[:, pr, :],
                        t16[:, pr.start + 1 : pr.stop + 1, :],
                        add,
                    )
                    if j == 3:
                        nc.vector.tensor_tensor(
                            Sv[:, 63:64, :], t16[:, 63:64, :], t16[:, 0:1, :], add
                        )
                    # ACT: A = 2*x8, V = 2*Sv (scaled copies)
                    nc.scalar.mul(o[:, ah, a_cols], x8[:, hr, :], 2.0)
                    nc.scalar.mul(o[:, vd, v_cols], Sv[:, pr, :], 2.0)
                    # DVE: H = x8[k]+x8[k+1], D = Sv[k]+Sv[k+1]
                    nc.vector.tensor_tensor(
                        o[:, ah, hm_cols], x8[:, hr, 0:63], x8[:, hr, 1:64], add
                    )
                    nc.vector.tensor_tensor(
                        o[:, ah, hw_col : hw_col + 1],
                        x8[:, hr, 63:64],
                        x8[:, hr, 0:1],
                        add,
                    )
                    nc.vector.tensor_tensor(
                        o[:, vd, dm_cols], Sv[:, pr, 0:63], Sv[:, pr, 1:64], add
                    )
                    nc.vector.tensor_tensor(
                        o[:, vd, dw_col : dw_col + 1],
                        Sv[:, pr, 63:64],
                        Sv[:, pr, 0:1],
                        add,
                    )
                    if j == 3:
                        # wrap row (pair 63) at local row 32 of o_3
                        wr = slice(32, 33)
                        pw = slice(63, 64)
                        nc.scalar.mul(o[:, wr, v_cols], Sv[:, pw, :], 2.0)
                        nc.vector.tensor_tensor(
                            o[:, wr, dm_cols], Sv[:, pw, 0:63], Sv[:, pw, 1:64], add
                        )
                        nc.vector.tensor_tensor(
                            o[:, wr, dw_col : dw_col + 1],
                            Sv[:, pw, 63:64],
                            Sv[:, pw, 0:1],
                            add,
                        )

            # o-tile row spans (before r shift): q0 31 rows, q1/q2 32, q3 33.
            O_ROWS = (31, 32, 32, 33)

            for ci in range(len(chunks)):
                b, h = chunks[ci]
                xs = all_xs[ci]
                t16 = t16p.tile([128, N, N], bf16, tag="t16")
                x8 = x8p.tile([128, N, N], bf16, tag="x8", name=f"x8_{b}_{h}")
                Sv = svp.tile([128, N, N], bf16, tag="sv", name=f"sv_{b}_{h}")
                os = [
                    op.tile([128, O_ROWS[j], M], f32, tag=f"o{j}", name=f"o_{b}_{h}_{j}")
                    for j in range(4)
                ]
                cv = val[(b, "cv")]
                with tc.If(cv < 1) as cmp:
                    compute_chunk(ci, xs, t16, x8, Sv, os, 0)
                with cmp.Else():
                    compute_chunk(ci, xs, t16, x8, Sv, os, 1)

                out3 = out[b, 128 * h : 128 * (h + 1)]  # [128ch, 128, 128]
                rv = val[(b, "rv")]
                # Contiguous-row stores; static APs in both arms so Tile
                # proves row-disjointness and the stores drain in parallel.
                # Each store is partition-split into two 64-partition
                # transfers (4 DGE stripes each instead of 8): the final
                # stripe of a transfer carries its completion overhead, and
                # with 8-stripe transfers that overhead always lands on
                # SDMA engines 7/15 (the stripe rotation advances by 8);
                # 4-stripe transfers rotate it across 3/7/11/15.
                def store2(dst, srctile, rows):
                    nc.sync.dma_start(dst[0:64, :, :], srctile[0:64, rows, :])
                    nc.sync.dma_start(dst[64:128, :, :], srctile[64:128, rows, :])

                with tc.If(rv < 1) as smp:
                    store2(out3[:, 0:31, :], os[0], slice(0, 31))
                    store2(out3[:, 31:63, :], os[1], slice(0, 32))
                    store2(out3[:, 63:95, :], os[2], slice(0, 32))
                    store2(out3[:, 95:128, :], os[3], slice(0, 33))
                with smp.Else():
                    store2(out3[:, 1:32, :], os[0], slice(0, 31))
                    store2(out3[:, 32:64, :], os[1], slice(0, 32))
                    store2(out3[:, 64:96, :], os[2], slice(0, 32))
                    store2(out3[:, 96:128, :], os[3], slice(0, 32))
                    store2(out3[:, 0:1, :], os[3], slice(32, 33))
    return nc


def _get_nc():
    global _NC_CACHE
    if _NC_CACHE is None:
        _NC_CACHE = _build_nc()
    return _NC_CACHE


def _offsets_for(idx_pair):
    offs = np.zeros((1, 16), np.int32)
    for j, p in enumerate(idx_pair):
        p = int(p)
        r, c = p % 2, p // 2
        offs[0, 8 * j : 8 * j + 4] = (c, r, 64 + r, (127 + r) % 128)
    return offs


def _to_np(a):
    if isinstance(a, np.ndarray):
        return a
    try:
        return np.asarray(a)
    except Exception:
        import jax

        return np.asarray(jax.device_put(a, jax.devices("cpu")[0]))


def kernel(inp, polyphase_indices, _trace=False):
    from concourse.bass_utils import run_bass_kernel_spmd

    inp = np.ascontiguousarray(_to_np(inp), dtype=np.float32)
    idx = _to_np(polyphase_indices).astype(np.int32).reshape(B)
    assert inp.shape == (B, C, N, N)

    in_maps = []
    for k in range(NCORES):
        in_maps.append(
            {
                "inp": np.ascontiguousarray(inp[NB * k : NB * (k + 1)]),
                "offs": _offsets_for(idx[NB * k : NB * (k + 1)]),
            }
        )

    nc = _get_nc()
    if not nc.is_finalized():
        nc.finalize()
    res = run_bass_kernel_spmd(
        nc, in_maps, core_ids=list(range(NCORES)), trace=_trace
    )
    out = np.concatenate([res.results[k]["out"] for k in range(NCORES)], axis=0)
    if _trace:
        kernel.last_results = res
    return out


# revision 40
# speedup vs baseline: 1.0242x; 1.0242x over previous
"""Polyphase 2x upsample (scatter into one of 4 phases per batch) + circular
3x3 binomial blur, distributed over 8 TRN2 NeuronCores (data-parallel over
batch: 2 batches per core).

Math: with phase p per batch, r = p % 2, c = p // 2, the reference scatters
x[i,j] to y1[2i+r, 2j+c] (zeros elsewhere) and then blurs with
outer([1,2,1],[1,2,1])/16 under circular padding. The output decomposes into
4 parity classes (all indices mod 128, mod 64 inside a pair):
  out[2i+r,   2j+c]   = x[i,j] / 4                    (A sites)
  out[2i+r,   2k+1+c] = (x[i,k] + x[i,k+1]) / 8       (H sites)
  out[2i+1+r, 2j+c]   = (x[i,j] + x[i+1,j]) / 8       (V sites)
  out[2i+1+r, 2k+1+c] = sum of the 4 neighbours / 16  (D sites)
All multiplies are powers of two (exact in fp32).

Memory-bound: 40 MiB/core of HBM traffic (8 read + 32 write), reads and
writes sharing one ~365 GB/s per-core cap => ~116us of saturated DMA is the
floor, plus ~9us of fixed NEFF/engine-init preamble. Schedule principles
(all measured on trn2):
 - Quarter-granularity store pipeline: each (batch, channel-half) chunk's
   128 output rows are produced in four ~32-row tiles, each stored the
   moment its sites complete; the first store issues at ~13us instead of
   ~37us (whole-chunk granularity), which removes the DMA hole between the
   end of the input-load stream and the first store.
 - Input loads are compressed into the FIRST ~25us (chunk 0's on ACT,
   chunks 1-2's upfront on SP, chunk 3's prefetched mid-chunk-1 from ACT):
   sustained load/store overlap skews SDMA ring 15 ~15% slow (its AXI port
   also serves DGE descriptor traffic), which shows up as a multi-us solo
   ring-15 drain tail after every balanced ring has finished. Keeping the
   overlap window short keeps ring 15's excess small. (Measured: overlap
   across the whole kernel costs ring 15 ~+15us of busy time regardless of
   load queue (SP vs ACT), store count (12 vs 18), or store shape
   (31/32/33 vs 64/63/1-row).)
 - SP issues only stores (plus the c1/c2 load issues that complete before
   the first store is data-ready): a dma_start costs ~850ns of issue time
   on its queue, so 16 load issues ahead of the store If would push the
   first store out by ~14us.
 - Strided-row DMA stores (per-row 512B descriptors) cost ~36% more HBM
   time than contiguous stores; all stores are contiguous row ranges.
 - GPSIMD software tensor ops contend with DVE for SBUF; Pool does no
   compute here. tensor_tensor_reduce faults the runtime; use adds.
 - HWDGE dma_start exists only on SP and ACT queues.

SPMD phase handling (one NEFF for all 8 cores):
 - The column phase bit c selects between two fully static write layouts
   via a runtime 2-arm If per chunk. All tiles are allocated OUTSIDE the
   If; both arms touch the same tiles with identical op counts. Pool-slot
   recycling must only happen ACROSS Ifs (slot release accounting for
   readers inside If arms reconciles at the If merge; reacquiring within
   the same If deadlocks).
 - The row shift r is folded into the output DMA's DRAM row offsets via a
   2-arm If on SP: static starts in both arms, so Tile proves all stores
   of a chunk hit disjoint DRAM rows and they drain in parallel.
 - skip_runtime_bounds_check everywhere: the emitted software assert
   instruction faults this runtime.
"""

import sys

for _p in ("/opt/trn_rl_repo",):
    if _p not in sys.path:
        sys.path.insert(0, _p)

import numpy as np

B, C, N = 16, 256, 64
M = 2 * N
NCORES = 8
NB = B // NCORES  # batches per core

_NC_CACHE = None


def _build_nc():
    import concourse.bacc as bacc
    import concourse.bass as bass
    import concourse.mybir as mybir
    import concourse.tile as tile

    f32 = mybir.dt.float32
    bf16 = mybir.dt.bfloat16
    i32 = mybir.dt.int32
    add = mybir.AluOpType.add
    ET = mybir.EngineType

    # Bacc (not plain Bass): its finalize() runs generate_event_semaphores,
    # which splits multi-wait instructions — this walrus build allows at
    # most one attached semaphore wait per instruction.
    nc = bacc.Bacc("TRN2", target_bir_lowering=False, debug=False, num_devices=NCORES)
    inp = nc.dram_tensor("inp", [NB, C, N, N], f32, kind="ExternalInput")
    offs = nc.dram_tensor("offs", [1, 16], i32, kind="ExternalInput")
    out = nc.dram_tensor("out", [NB, C, M, M], f32, kind="ExternalOutput")

    chunks = [(b, h) for b in range(NB) for h in range(C // 128)]

    with tile.TileContext(nc) as tc:
        with (
            tc.tile_pool(name="offp", bufs=1) as offp,
            tc.tile_pool(name="xp", bufs=12) as xp,
            tc.tile_pool(name="t16p", bufs=1) as t16p,
            tc.tile_pool(name="x8p", bufs=1) as x8p,
            tc.tile_pool(name="svp", bufs=1) as svp,
            tc.tile_pool(name="op", bufs=2) as op,
        ):
            def alloc_x(ci):
                b, h = chunks[ci]
                return [
                    xp.tile([128, 16, N], f32, tag="x", name=f"x_{b}_{h}_{j}")
                    for j in range(4)
                ]

            def issue_loads(ci, tiles, eng):
                b, h = chunks[ci]
                for j in range(4):
                    eng.dma_start(
                        tiles[j][:, :, :],
                        inp[b, 128 * h : 128 * (h + 1), 16 * j : 16 * j + 16],
                    )

            # ACT: offs (tiny) then chunk 0's loads, then the cv reg-loads
            # (which block ACT until offs lands — the input loads must
            # already be in flight). SP: chunks 1-2's loads upfront (done
            # issuing by ~13us, before the first store is data-ready),
            # then the rv reg-loads.
            offs_t = offp.tile([1, 16], i32)
            nc.scalar.dma_start(offs_t[:, :], offs[:, :])
            all_xs = [alloc_x(ci) for ci in range(len(chunks))]
            issue_loads(0, all_xs[0], nc.scalar)
            issue_loads(1, all_xs[1], nc.sync)
            issue_loads(2, all_xs[2], nc.sync)

            # per batch: [cv, rv] at offs[0, 8*b + k]
            val = {}
            for b in range(NB):
                for k, name, engs in (
                    (0, "cv", (ET.DVE, ET.Activation)),
                    (1, "rv", (ET.SP, ET.Activation)),
                ):
                    val[(b, name)] = nc.values_load(
                        offs_t[0:1, 8 * b + k : 8 * b + k + 1],
                        engines=list(engs),
                        min_val=0,
                        max_val=1,
                        skip_runtime_bounds_check=True,
                    )

            # Per-quarter output row groups (output row index before r shift):
            #   q0 -> rows [0,31)   : A/H at local even rows, V/D odd
            #   q1 -> rows [31,63)  : V/D at local even rows, A/H odd
            #   q2 -> rows [63,95)  : V/D even, A/H odd
            #   q3 -> rows [95,128) : V/D even, A/H odd, local row 32 = pair 63
            # Quarter j's A/H rows read x8 rows [16j,16j+16); its V/D rows
            # read Sv pairs (q0: [0,15), q1: [15,31), q2: [31,47),
            # q3: [47,63) plus the wrap pair 63 at Sv row 63). These spans
            # only need t16 rows <= 16j+15, so no quarter waits on a later
            # load.
            def compute_chunk(ci, xs, t16, x8, Sv, os, c):
                if c == 0:
                    a_cols = slice(0, 128, 2)
                    hm_cols = slice(1, 127, 2)
                    hw_col = 127
                    v_cols = slice(0, 128, 2)
                    dm_cols = slice(1, 127, 2)
                    dw_col = 127
                else:
                    a_cols = slice(1, 128, 2)
                    hm_cols = slice(2, 127, 2)
                    hw_col = 0
                    v_cols = slice(1, 128, 2)
                    dm_cols = slice(2, 127, 2)
                    dw_col = 0
                for j in range(4):
                    if ci == 1 and j == 2:
                        # chunk 3's loads: issued mid-chunk-1 from ACT; the
                        # issue's WAR conflict (xp bufs=12) is chunk 0's
                        # tiles, whose readers ran in chunk 0's If — one If
                        # back, so the slot accounting has reconciled.
                        issue_loads(3, all_xs[3], nc.scalar)
                    xq, o = xs[j], os[j]
                    hr = slice(16 * j, 16 * j + 16)
                    # t16 = x/16 feeds Sv (built on DVE to leave ACT
                    # issue-time for the o1/o3 store DMAs); x8 = x/8 feeds
                    # A and H.
                    nc.vector.tensor_scalar_mul(t16[:, hr, :], xq[:, :, :], 0.0625)
                    nc.scalar.mul(x8[:, hr, :], xq[:, :, :], 0.125)
                    # Sv pairs needed by this quarter's V/D rows
                    if j == 0:
                        pr = slice(0, 15)
                        ah = slice(0, 31, 2)   # 16 rows
                        vd = slice(1, 30, 2)   # 15 rows
                    else:
                        pr = slice(16 * j - 1, 16 * j + 15)
                        ah = slice(1, 32, 2)   # 16 rows
                        vd = slice(0, 31, 2)   # 16 rows
                    nc.vector.tensor_tensor(
                        Sv[:, pr, :],
                        t16[:, pr, :],
                        t16[:, pr.start + 1 : pr.stop + 1, :],
                        add,
                    )
                    if j == 3:
                        nc.vector.tensor_tensor(
                            Sv[:, 63:64, :], t16[:, 63:64, :], t16[:, 0:1, :], add
                        )
                    # ACT: A = 2*x8, V = 2*Sv (scaled copies)
                    nc.scalar.mul(o[:, ah, a_cols], x8[:, hr, :], 2.0)
                    nc.scalar.mul(o[:, vd, v_cols], Sv[:, pr, :], 2.0)
                    # DVE: H = x8[k]+x8[k+1], D = Sv[k]+Sv[k+1]
                    nc.vector.tensor_tensor(
                        o[:, ah, hm_cols], x8[:, hr, 0:63], x8[:, hr, 1:64], add
                    )
                    nc.vector.tensor_tensor(
                        o[:, ah, hw_col : hw_col + 1],
                        x8[:, hr, 63:64],
                        x8[:, hr, 0:1],
                        add,
                    )
                    nc.vector.tensor_tensor(
                        o[:, vd, dm_cols], Sv[:, pr, 0:63], Sv[:, pr, 1:64], add
                    )
                    nc.vector.tensor_tensor(
                        o[:, vd, dw_col : dw_col + 1],
                        Sv[:, pr, 63:64],
                        Sv[:, pr, 0:1],
                        add,
                    )
                    if j == 3:
                        # wrap row (pair 63) at local row 32 of o_3
                        wr = slice(32, 33)
                        pw = slice(63, 64)
                        nc.scalar.mul(o[:, wr, v_cols], Sv[:, pw, :], 2.0)
                        nc.vector.tensor_tensor(
                            o[:, wr, dm_cols], Sv[:, pw, 0:63], Sv[:, pw, 1:64], add
                        )
                        nc.vector.tensor_tensor(
                            o[:, wr, dw_col : dw_col + 1],
                            Sv[:, pw, 63:64],
                            Sv[:, pw, 0:1],
                            add,
                        )

            # o-tile row spans (before r shift): q0 31 rows, q1/q2 32, q3 33.
            O_ROWS = (31, 32, 32, 33)

            for ci in range(len(chunks)):
                b, h = chunks[ci]
                xs = all_xs[ci]
                t16 = t16p.tile([128, N, N], bf16, tag="t16")
                x8 = x8p.tile([128, N, N], bf16, tag="x8", name=f"x8_{b}_{h}")
                Sv = svp.tile([128, N, N], bf16, tag="sv", name=f"sv_{b}_{h}")
                os = [
                    op.tile([128, O_ROWS[j], M], f32, tag=f"o{j}", name=f"o_{b}_{h}_{j}")
                    for j in range(4)
                ]
                cv = val[(b, "cv")]
                with tc.If(cv < 1) as cmp:
                    compute_chunk(ci, xs, t16, x8, Sv, os, 0)
                with cmp.Else():
                    compute_chunk(ci, xs, t16, x8, Sv, os, 1)

                out3 = out[b, 128 * h : 128 * (h + 1)]  # [128ch, 128, 128]
                rv = val[(b, "rv")]
                # Contiguous-row stores; static APs in both arms so Tile
                # proves row-disjointness and the stores drain in parallel.
                # The stores are split across BOTH HWDGE queues (o1/o3 on
                # ACT, o0/o2 on SP): a single queue's stripe rotation puts
                # every transfer's completion-carrying final stripe on the
                # same pair of SDMA engines, and the resulting ring-15
                # backlog drains solo for ~10us after every other ring has
                # finished. o1/o3's data is complete by the time the ACT
                # queue reaches these issues (end of the chunk's compute),
                # so they block ACT only momentarily.
                with tc.If(rv < 1) as amp:
                    nc.scalar.dma_start(out3[:, 31:63, :], os[1][:, :, :])
                    nc.scalar.dma_start(out3[:, 95:128, :], os[3][:, :, :])
                with amp.Else():
                    nc.scalar.dma_start(out3[:, 32:64, :], os[1][:, :, :])
                    nc.scalar.dma_start(out3[:, 96:128, :], os[3][:, 0:32, :])
                    nc.scalar.dma_start(out3[:, 0:1, :], os[3][:, 32:33, :])
                with tc.If(rv < 1) as smp:
                    nc.sync.dma_start(out3[:, 0:31, :], os[0][:, :, :])
                    nc.sync.dma_start(out3[:, 63:95, :], os[2][:, :, :])
                with smp.Else():
                    nc.sync.dma_start(out3[:, 1:32, :], os[0][:, :, :])
                    nc.sync.dma_start(out3[:, 64:96, :], os[2][:, :, :])
    return nc


def _get_nc():
    global _NC_CACHE
    if _NC_CACHE is None:
        _NC_CACHE = _build_nc()
    return _NC_CACHE


def _offsets_for(idx_pair):
    offs = np.zeros((1, 16), np.int32)
    for j, p in enumerate(idx_pair):
        p = int(p)
        r, c = p % 2, p // 2
        offs[0, 8 * j : 8 * j + 4] = (c, r, 64 + r, (127 + r) % 128)
    return offs


def _to_np(a):
    if isinstance(a, np.ndarray):
        return a
    try:
        return np.asarray(a)
    except Exception:
        import jax

        return np.asarray(jax.device_put(a, jax.devices("cpu")[0]))


def kernel(inp, polyphase_indices, _trace=False):
    from concourse.bass_utils import run_bass_kernel_spmd

    inp = np.ascontiguousarray(_to_np(inp), dtype=np.float32)
    idx = _to_np(polyphase_indices).astype(np.int32).reshape(B)
    assert inp.shape == (B, C, N, N)

    in_maps = []
    for k in range(NCORES):
        in_maps.append(
            {
                "inp": np.ascontiguousarray(inp[NB * k : NB * (k + 1)]),
                "offs": _offsets_for(idx[NB * k : NB * (k + 1)]),
            }
        )

    nc = _get_nc()
    if not nc.is_finalized():
        nc.finalize()
    res = run_bass_kernel_spmd(
        nc, in_maps, core_ids=list(range(NCORES)), trace=_trace
    )
    out = np.concatenate([res.results[k]["out"] for k in range(NCORES)], axis=0)
    if _trace:
        kernel.last_results = res
    return out
